# revision 18
# baseline (speedup 1.0000x reference)
"""Trainium2 Bass kernel for ContrastHead (softnn contrastive KNN loss).

Descriptor-minimized neighbor gather. The workload is bound by SWDGE gather-
descriptor generation on the GPSIMD Q7 DSPs (~8.5ns/descriptor per queue;
queue q runs on DSP pair 2q/2q+1, so 4 SWDGE queues engage all 8 DSPs at
~2.1ns/desc aggregate). Design:

- Table rows pair-pack TWO points per 256B row (features only, f16), so int16
  gather indices need just 2 windows (<=32767 rows each) instead of 4.
- posmask is computed on the host from labels (labels are never gathered);
  dist^2 = sum((f_i - f_j)^2) so norms are never gathered either.
- Per core, points are sorted by window-0 neighbor count c0 and packed
  128/tile; per tile the w0 gather fills slots [0, k0) and the w1 gather
  fills [k0, k0+k1) of one buffer - disjoint ranges, no OR-merge, ragged
  per-tile slot counts (~1% padding): ~393k descriptors/core vs 1.55M for
  the naive 4-window scheme. Gathers round-robin the 4 SWDGE queues.
  Dummy row = 30.0^64 => dist ~ 240 => exp = 0, so pad slots vanish from
  the softmax; unwritten ragged slots are pre-memset to 1e6 in dist^2.
- Parity (which half of the pair row) is one DVE copy_predicated on i32
  views; then subtract/square/tree-add/reduce -> dist^2 per slot.
- Phase 2 is split into blocks (tiles 0-59, 60-94, 95-97): the bulk reduces
  to masked log-ratio sums while later gathers still run, so only a 3-tile
  remainder lands in the serial tail. The cnt-mask compares use tensor_tensor
  with broadcast consts (tensor_scalar's compare path costs ~23us per op).
- The idx upload is chunked 8x so the first gather starts immediately.
Host sums the 8x(128,6) outputs: loss = -(sum lg)/max(cnt,1).
"""

import numpy as np

import concourse.bacc as bacc
import concourse.bass as bass
import concourse.mybir as mybir
import concourse.tile as tile
from concourse import bass_utils

F16 = mybir.dt.float16
F32 = mybir.dt.float32
I16 = mybir.dt.int16
I32 = mybir.dt.int32

N = 100000
K = 31
C = 64
NPAIR = N // 2              # 50000 pair rows, 256B each
WINR = 32766                # real pair-rows per window
WSTR = WINR + 1             # window stride (incl dummy row 0)
NWIN = 2
NCORES = 8
PTS = N // NCORES           # 12500
TPC = (PTS + 127) // 128    # 98 tiles/core
PADPTS = TPC * 128          # 12544
TEMP = 0.1
EPS = 1e-8
DUMMY = 30.0                # dummy-row feature value -> dist ~240 -> exp 0

_CACHE = {}


def _build(k0g, k1g):
    """k0g/k1g: per-tile window-0/1 slot counts (len TPC), cross-core maxes."""
    nc = bacc.Bacc("TRN2", target_bir_lowering=False, debug=False,
                   num_swdge_queues=4)
    kp = [a + b for a, b in zip(k0g, k1g)]
    kmax = max(kp)
    tot16 = 8 * sum(kp)
    qa_of = [t % 4 for t in range(TPC)]
    qb_of = [(t + 2) % 4 for t in range(TPC)]

    tabT = nc.dram_tensor("tab", (NWIN * WSTR, 128), F16, kind="ExternalInput")
    selfT = nc.dram_tensor("selftab", (128, TPC, C), F16, kind="ExternalInput")
    bounds = [0]
    for a, b in zip(k0g, k1g):
        bounds.append(bounds[-1] + 8 * (a + b))     # idx col offset per tile
    tile_ch = [0, 2, 15, 28, 41, 54, 67, 80, TPC]   # chunk tile boundaries
    splits = [bounds[b] for b in tile_ch]
    ch_of = []
    for ci in range(8):
        ch_of += [ci] * (tile_ch[ci + 1] - tile_ch[ci])
    idxT = nc.dram_tensor("nidx16", (128, tot16), I16,
                          kind="ExternalInput")
    parT = nc.dram_tensor("par", (128, TPC, kmax), I16, kind="ExternalInput")
    pmT = nc.dram_tensor("pm", (128, TPC, kmax), F32, kind="ExternalInput")
    outT = nc.dram_tensor("out", (128, 6), F32, kind="ExternalOutput")
    BL = [(0, 60), (60, 95), (95, TPC)]              # phase-2 blocks

    with tile.TileContext(nc) as tc:
        with (
            tc.tile_pool(name="res", bufs=1) as res,
            tc.tile_pool(name="gpool", bufs=8) as gpool,
            tc.tile_pool(name="mpool", bufs=2) as mpool,
            tc.tile_pool(name="p2", bufs=1) as p2,
        ):
            idxchunks = []
            for ci in range(8):
                a, b = splits[ci], splits[ci + 1]
                ch = res.tile([128, b - a], I16, tag=f"idx{ci}")
                nc.sync.dma_start(out=ch[:], in_=idxT.ap()[:, a:b])
                idxchunks.append((a, ch))
            parsb = res.tile([128, TPC, kmax], I16)
            nc.sync.dma_start(out=parsb[:], in_=parT.ap())
            selfsb = res.tile([128, TPC, C], F16)
            nc.sync.dma_start(out=selfsb[:], in_=selfT.ap())
            pmsb = res.tile([128, TPC, kmax], F32)
            nc.sync.dma_start(out=pmsb[:], in_=pmT.ap())

            d2blocks = []
            for bi, (lo, hi) in enumerate(BL):
                d2 = res.tile([128, hi - lo, kmax], F32, tag=f"d2_{bi}")
                nc.vector.memset(d2[:], 1.0e6)
                d2blocks.append(d2)
            eps_t = p2.tile([128, 1], F32)
            nc.vector.memset(eps_t[:], EPS)
            chalf = p2.tile([128, 1], F32)
            nc.vector.memset(chalf[:], 0.5)
            ckm = p2.tile([128, 1], F32)
            nc.vector.memset(ckm[:], float(K) - 0.5)
            outsb = p2.tile([128, 6], F32)

            def do_tile(t):
                k0 = k0g[t]
                k1 = k1g[t]
                kpt = k0 + k1
                base = bounds[t]
                ci = ch_of[t]
                choff, idxsb = idxchunks[ci]
                base -= choff
                g = gpool.tile([128, kmax, 128], F16, tag="g")
                nc.gpsimd.dma_gather(
                    out_ap=g[:, 0:k0, :],
                    in_ap=tabT.ap()[0:WSTR, :],
                    idxs_ap=idxsb[:, base : base + 8 * k0],
                    num_idxs=128 * k0,
                    num_idxs_reg=128 * k0,
                    elem_size=128,
                    single_packet=False,
                    queue_num=qa_of[t],
                )
                nc.gpsimd.dma_gather(
                    out_ap=g[:, k0:kpt, :],
                    in_ap=tabT.ap()[WSTR : 2 * WSTR, :],
                    idxs_ap=idxsb[:, base + 8 * k0 : base + 8 * kpt],
                    num_idxs=128 * k1,
                    num_idxs_reg=128 * k1,
                    elem_size=128,
                    single_packet=False,
                    queue_num=qb_of[t],
                )
                # parity select: overwrite half A with half B where par != 0
                pbc = parsb[:, t, 0:kpt].unsqueeze(2).broadcast_to(
                    [128, kpt, C // 2]
                )
                nc.vector.copy_predicated(
                    out=g[:, 0:kpt, 0:C].bitcast(I32),
                    mask=pbc,
                    data=g[:, 0:kpt, C : 2 * C].bitcast(I32),
                )
                d = mpool.tile([128, kmax, C], F16, tag="d")
                fb = selfsb[:, t, :].unsqueeze(1).broadcast_to([128, kpt, C])
                nc.vector.tensor_tensor(
                    out=d[:, 0:kpt, :], in0=g[:, 0:kpt, 0:C], in1=fb,
                    op=mybir.AluOpType.subtract,
                )
                nc.vector.tensor_tensor(
                    out=d[:, 0:kpt, :], in0=d[:, 0:kpt, :], in1=d[:, 0:kpt, :],
                    op=mybir.AluOpType.mult,
                )
                nc.vector.tensor_add(
                    out=d[:, 0:kpt, 0:32], in0=d[:, 0:kpt, 0:32],
                    in1=d[:, 0:kpt, 32:64],
                )
                nc.vector.tensor_add(
                    out=d[:, 0:kpt, 0:16], in0=d[:, 0:kpt, 0:16],
                    in1=d[:, 0:kpt, 16:32],
                )
                nc.vector.tensor_add(
                    out=d[:, 0:kpt, 0:8], in0=d[:, 0:kpt, 0:8],
                    in1=d[:, 0:kpt, 8:16],
                )
                bi = next(i for i, (lo, hi) in enumerate(BL) if t < hi)
                nc.vector.reduce_sum(
                    out=d2blocks[bi][:, t - BL[bi][0], 0:kpt],
                    in_=d[:, 0:kpt, 0:8],
                    axis=mybir.AxisListType.X,
                )

            def phase2(dist2, lo, hi, col):
                nt = hi - lo
                pms = pmsb[:, lo:hi, :]
                nc.scalar.sqrt(out=dist2[:], in_=dist2[:])
                mind = p2.tile([128, nt], F32, tag=f"mind{col}")
                nc.vector.tensor_reduce(
                    out=mind[:], in_=dist2[:], axis=mybir.AxisListType.X,
                    op=mybir.AluOpType.min,
                )
                mbc = mind[:].unsqueeze(2).broadcast_to([128, nt, kmax])
                nc.vector.tensor_tensor(
                    out=dist2[:], in0=dist2[:], in1=mbc,
                    op=mybir.AluOpType.subtract,
                )
                nc.scalar.activation(
                    out=dist2[:], in_=dist2[:],
                    func=mybir.ActivationFunctionType.Exp, scale=-1.0 / TEMP,
                )
                negs = p2.tile([128, nt], F32, tag=f"negs{col}")
                nc.vector.reduce_sum(
                    out=negs[:], in_=dist2[:], axis=mybir.AxisListType.X
                )
                nc.vector.tensor_tensor(
                    out=dist2[:], in0=dist2[:], in1=pms,
                    op=mybir.AluOpType.mult,
                )
                poss = p2.tile([128, nt], F32, tag=f"poss{col}")
                nc.vector.reduce_sum(
                    out=poss[:], in_=dist2[:], axis=mybir.AxisListType.X
                )
                cnts = p2.tile([128, nt], F32, tag=f"cnts{col}")
                nc.vector.reduce_sum(
                    out=cnts[:], in_=pms, axis=mybir.AxisListType.X
                )
                rn = p2.tile([128, nt], F32, tag=f"rn{col}")
                nc.vector.reciprocal(out=rn[:], in_=negs[:])
                ratio = p2.tile([128, nt], F32, tag=f"ratio{col}")
                nc.vector.tensor_tensor(
                    out=ratio[:], in0=poss[:], in1=rn[:],
                    op=mybir.AluOpType.mult,
                )
                lg = p2.tile([128, nt], F32, tag=f"lg{col}")
                nc.scalar.activation(
                    out=lg[:], in_=ratio[:],
                    func=mybir.ActivationFunctionType.Ln, bias=eps_t[:],
                )
                ma = p2.tile([128, nt], F32, tag=f"ma{col}")
                nc.vector.tensor_tensor(
                    out=ma[:], in0=cnts[:],
                    in1=chalf[:].broadcast_to([128, nt]),
                    op=mybir.AluOpType.is_gt,
                )
                mb2 = p2.tile([128, nt], F32, tag=f"mb2{col}")
                nc.vector.tensor_tensor(
                    out=mb2[:], in0=cnts[:],
                    in1=ckm[:].broadcast_to([128, nt]),
                    op=mybir.AluOpType.is_lt,
                )
                nc.vector.tensor_tensor(
                    out=ma[:], in0=ma[:], in1=mb2[:], op=mybir.AluOpType.mult
                )
                nc.vector.tensor_tensor(
                    out=lg[:], in0=lg[:], in1=ma[:], op=mybir.AluOpType.mult
                )
                nc.vector.reduce_sum(
                    out=outsb[:, col : col + 1], in_=lg[:],
                    axis=mybir.AxisListType.X,
                )
                nc.vector.reduce_sum(
                    out=outsb[:, col + 1 : col + 2], in_=ma[:],
                    axis=mybir.AxisListType.X,
                )

            for bi, (lo, hi) in enumerate(BL):
                for t in range(lo, hi):
                    do_tile(t)
                phase2(d2blocks[bi], lo, hi, 2 * bi)
            nc.sync.dma_start(out=outT.ap(), in_=outsb[:])

    nc.compile()
    return nc


def _pack_table(features: np.ndarray) -> np.ndarray:
    pairs = features.astype(np.float16).reshape(NPAIR, 2 * C)
    tab = np.zeros((NWIN * WSTR, 2 * C), dtype=np.float16)
    tab[0] = DUMMY
    tab[WSTR] = DUMMY
    tab[1 : 1 + WINR] = pairs[0:WINR]
    tab[WSTR + 1 : WSTR + 1 + (NPAIR - WINR)] = pairs[WINR:NPAIR]
    return tab


def _wrap_idx(vals):
    """vals (128, kcols) slot-major per partition -> SWDGE int16 layout."""
    n = vals.shape[1] * 128
    flat = vals.T.reshape(n)                       # slot i = j*128 + p
    wrapped = flat.reshape(n // 16, 16).T          # (16, n/16)
    return np.tile(wrapped, (8, 1)).astype(np.int16)


def _prep_core(features, neighbor_idx, posmask, lo, hi):
    """Per-core sorted/split neighbor metadata (before global K sizing)."""
    nbr = neighbor_idx[lo:hi].astype(np.int64)     # (PTS, K)
    prow = nbr >> 1
    par = (nbr & 1).astype(np.int16)
    inw1 = prow >= WINR
    c0 = (~inw1).sum(axis=1).astype(np.int64)      # (PTS,)
    order = np.argsort(c0, kind="stable")

    # neighbor-sorted arrays, padded to PADPTS (pads: c0=31, no real slots)
    perm = np.argsort(inw1[order], axis=1, kind="stable")
    prow_s = np.take_along_axis(prow[order], perm, axis=1)
    par_s = np.take_along_axis(par[order], perm, axis=1)
    pos_s = np.take_along_axis(posmask[lo:hi][order], perm, axis=1)
    c0_s = c0[order]

    pad = PADPTS - (hi - lo)
    prow_s = np.concatenate([prow_s, np.zeros((pad, K), np.int64)])
    par_s = np.concatenate([par_s, np.zeros((pad, K), np.int16)])
    pos_s = np.concatenate([pos_s, np.zeros((pad, K), np.float32)])
    c0_s = np.concatenate([c0_s, np.full(pad, K, np.int64)])
    nreal = np.concatenate(
        [np.full(hi - lo, K, np.int64), np.zeros(pad, np.int64)]
    )
    c0_s = np.where(nreal == 0, K, c0_s)           # pads: all-w0, zero real
    feat_s = np.concatenate(
        [features[lo:hi].astype(np.float16)[order],
         np.zeros((pad, C), np.float16)]
    )
    return prow_s, par_s, pos_s, c0_s, nreal, feat_s


def _core_inputs(table, prep, k0g, k1g):
    prow_s, par_s, pos_s, c0_s, nreal, feat_s = prep
    kp = [a + b for a, b in zip(k0g, k1g)]
    kmax = max(kp)
    idx_cols = np.zeros((128, 8 * sum(kp)), dtype=np.int16)
    par_t = np.zeros((128, TPC, kmax), dtype=np.int16)
    pm_t = np.zeros((128, TPC, kmax), dtype=np.float32)

    jj = np.arange(K)
    base = 0
    for t in range(TPC):
        sl = slice(t * 128, (t + 1) * 128)
        rows = prow_s[sl]                          # (128, K) sorted: w0 first
        pars = par_s[sl]
        poss = pos_s[sl]
        c0p = np.minimum(c0_s[sl], nreal[sl])      # real w0 count per point
        nre = nreal[sl]
        k0 = k0g[t]
        k1p = k1g[t]
        kpt = k0 + k1p

        m0 = jj[None, :k0] < c0p[:, None]          # (128, k0) real w0 slots
        idx0 = np.where(m0, rows[:, :k0] + 1, 0).astype(np.int16)
        par_t[:, t, :k0] = np.where(m0, pars[:, :k0], 0)
        pm_t[:, t, :k0] = np.where(m0, poss[:, :k0], 0)

        sidx = c0p[:, None] + np.arange(k1p)[None, :]   # (128, k1p)
        valid = sidx < nre[:, None]
        sc = np.clip(sidx, 0, K - 1)
        g1 = np.take_along_axis(rows, sc, axis=1)
        idx1 = np.where(valid, g1 - WINR + 1, 0).astype(np.int16)
        par_t[:, t, k0:kpt] = np.where(valid, np.take_along_axis(pars, sc, axis=1), 0)
        pm_t[:, t, k0:kpt] = np.where(valid, np.take_along_axis(poss, sc, axis=1), 0)

        idx_cols[:, base : base + 8 * k0] = _wrap_idx(idx0)
        idx_cols[:, base + 8 * k0 : base + 8 * kpt] = _wrap_idx(idx1)
        base += 8 * kpt

    return {
        "tab": table,
        "selftab": np.ascontiguousarray(
            feat_s.reshape(TPC, 128, C).transpose(1, 0, 2)
        ),
        "nidx16": idx_cols,
        "par": par_t,
        "pm": pm_t,
    }


def run(features, labels, neighbor_idx, trace=False):
    features = np.asarray(features)
    labels = np.asarray(labels)
    neighbor_idx = np.asarray(neighbor_idx)

    posmask = (labels[:, None] == labels[neighbor_idx]).astype(np.float32)
    table = _pack_table(features)

    preps = [
        _prep_core(features, neighbor_idx, posmask, c * PTS, (c + 1) * PTS)
        for c in range(NCORES)
    ]
    # global per-tile slot sizing (shared compiled kernel across cores)
    k0g = np.zeros(TPC, dtype=np.int64)
    k1g = np.zeros(TPC, dtype=np.int64)
    for prep in preps:
        c0_s, nreal = prep[3], prep[4]
        for t in range(TPC):
            sl = slice(t * 128, (t + 1) * 128)
            c0p = np.minimum(c0_s[sl], nreal[sl])
            k0g[t] = max(k0g[t], int(c0p.max()))
            k1g[t] = max(k1g[t], int((nreal[sl] - c0p).max()))
    k0g = [int(v) for v in k0g]
    k1g = [int(v) for v in k1g]

    key = (tuple(k0g), tuple(k1g))
    if _CACHE.get("key") != key:
        _CACHE["nc"] = _build(k0g, k1g)
        _CACHE["key"] = key
    nc = _CACHE["nc"]

    in_maps = [_core_inputs(table, preps[c], k0g, k1g) for c in range(NCORES)]
    res = bass_utils.run_bass_kernel_spmd(
        nc, in_maps, core_ids=list(range(NCORES)), trace=trace
    )
    s = 0.0
    ccnt = 0.0
    for o in res.results:
        out = o["out"].astype(np.float64)
        s += float(out[:, 0::2].sum())
        ccnt += float(out[:, 1::2].sum())
    loss = np.float32(-s / max(ccnt, 1.0))
    return loss, res


def kernel(features, labels, neighbor_idx):
    loss, _ = run(features, labels, neighbor_idx, trace=False)
    return loss
